# revision 1
# baseline (speedup 1.0000x reference)
"""Trainium2 Bass kernel for nn_PostProcessor (stereo NMS detection head).

Strategy (data-parallel over proposals, 8 cores):
  - Each core gets a contiguous shard of N/8 = 16384 proposals.
  - On device (per core): softmax scores + threshold mask, full box/center/
    dims/rot decode for foreground classes 1..3. All decoded features plus
    the masked score are written densely to a [16384, 3, 17] output.
  - On host: concatenate the 8 shards, then run the (tiny, ~120-deep) greedy
    stereo-NMS walk per class over score-sorted candidates, take the global
    top-100 and assemble the [100, 17] result — exactly replicating the
    reference's float32 semantics using the device-produced floats.

Feature layout per (proposal, class):
  d 0:4   boxes_left  (x1,y1,x2,y2)
  d 4:8   boxes_right
  d 8:10  centers_left
  d 10:12 centers_right
  d 12:15 dims (h,w,l)
  d 15    rot
  d 16    masked score (softmax score if > 0.05 else 0.0)
"""

import math
import sys

import numpy as np

for _p in ("/opt/trn_rl_repo", "/root/.axon_site/_ro/trn_rl_repo"):
    if _p not in sys.path:
        sys.path.insert(0, _p)

import concourse.bass as bass
import concourse.bacc as bacc
import concourse.tile as tile
from concourse import mybir
from concourse.bass_utils import run_bass_kernel_spmd

F32 = mybir.dt.float32
OP = mybir.AluOpType

NCORES = 8
N = 131072
NS = N // NCORES          # 16384 proposals per core
P = 128                   # SBUF partitions
FREE = NS // P            # 128 proposals per partition
CHUNK = 64                # proposals-per-partition per pipeline chunk
NCHUNK = FREE // CHUNK

C = 4                     # classes incl. background
NFG = C - 1               # foreground classes
B = 10                    # angle bins
D_FEAT = 17

IMG_W, IMG_H = 1280.0, 384.0
SCORE_THRESH = 0.05
NMS_THR = 0.5
MAX_PER_CLASS = 100
DETS_PER_IMG = 100
DW_CLAMP = math.log(1000.0 / 16.0)
EXP_CLAMP = float(np.float32(np.exp(DW_CLAMP)))   # exp of the clamp, f32
MEAN_DIMS = (1.53, 1.63, 3.88)
NEG = -1e30
BIN_SIZE = float(np.float32(2.0 * np.pi / B))
PI_F32 = float(np.float32(np.pi))

INPUT_SPECS = {
    "class_logits": C,
    "bbox_reg_left": 4 * C,
    "bbox_reg_right": 4 * C,
    "center_reg_left": 2 * C,
    "center_reg_right": 2 * C,
    "hwl_reg": 3 * C,
    "alpha_logit": B,
    "alpha_reg": C * B,
    "proposals_left": C,
    "proposals_right": C,
}# Packed input layouts (built host-side in _run_device), split by consumer
# group so each chunk's compute can start as soon as its own stream lands:
#   pack1 [NS, 14]: class_logits 0:4 | alpha_logit 4:14     (softmax + argmax)
#   pack2 [NS, 40]: alpha_reg, with class-0 bins (cols 0:10) overwritten by
#                   the bin-index constant 0..9 — so one eq*reg pass yields
#                   both the argmax label (c=0 lane) and the per-class
#                   residuals (c=1..3). Reference discards class-0 rots.
#   pack3 [NS, 8]:  proposals_left | proposals_right         (whole-shard)
#   pack4 [NS, 60]: bbox_l 0:16 | bbox_r 16:32 | ctr_l 32:40 | ctr_r 40:48
#                   | hwl 48:60
D1, D2, D3, D4 = 14, 40, 8, 60


def _build_nc():
    nc = bacc.Bacc("TRN2", target_bir_lowering=False, debug=False)

    dp1 = nc.declare_dram_parameter("pack1", [NS, D1], F32, isOutput=False)
    dp2 = nc.declare_dram_parameter("pack2", [NS, D2], F32, isOutput=False)
    dp3 = nc.declare_dram_parameter("pack3", [NS, D3], F32, isOutput=False)
    dp4 = nc.declare_dram_parameter("pack4", [NS, D4], F32, isOutput=False)
    dout = nc.declare_dram_parameter("feat", [NS, NFG, D_FEAT], F32, isOutput=True)

    # Partition-major views: proposal r -> partition r // FREE, slot r % FREE.
    v1 = dp1[:].rearrange("(p f) d -> p f d", p=P)
    v2 = dp2[:].rearrange("(p f) d -> p f d", p=P)
    v3 = dp3[:].rearrange("(p f) d -> p f d", p=P)
    v4 = dp4[:].rearrange("(p f) d -> p f d", p=P)
    vout = dout[:].rearrange("(p f) c d -> p f c d", p=P)

    AX = mybir.AxisListType.X
    EXP = mybir.ActivationFunctionType.Exp
    CPY = mybir.ActivationFunctionType.Copy

    with tile.TileContext(nc) as tc:
        with tc.tile_pool(name="pool", bufs=1) as pool:
            def MT(shape, tg):
                return pool.tile(shape, F32, tag=tg, name=tg)

            # proposals for the whole shard (both sides at once, f-major)
            props = MT([P, FREE, 2, 4], "props")
            nc.sync.dma_start(
                props[:], v3[:, :, :].rearrange("p f (s k) -> p f s k", s=2)
            )
            # wh = (p2 - p0) + 1, both coords & sides in one op: [P, F, 2s, 2k]
            wh = MT([P, FREE, 2, 2], "wh")
            nc.vector.tensor_tensor(
                wh[:], props[:, :, :, 2:4], props[:, :, :, 0:2], OP.subtract
            )
            nc.vector.tensor_scalar_add(wh[:], wh[:], 1.0)
            whh = MT([P, FREE, 2, 2], "whh")       # 0.5 * wh  (exact)
            nc.vector.tensor_scalar_mul(whh[:], wh[:], 0.5)
            wh01 = MT([P, FREE, 2, 2], "wh01")     # 0.1 * wh
            nc.vector.tensor_scalar_mul(wh01[:], wh[:], 0.1)
            cxy = MT([P, FREE, 2, 2], "cxy")       # x1 + 0.5*w , y1 + 0.5*h
            nc.vector.tensor_tensor(cxy[:], props[:, :, :, 0:2], whh[:], OP.add)

            for j in range(NCHUNK):
                s = slice(j * CHUNK, (j + 1) * CHUNK)

                def T(shape, tg):
                    return pool.tile(shape, F32, tag=f"{tg}_{j}", name=f"{tg}_{j}")

                p4 = T([P, CHUNK, D4], "p4")
                nc.sync.dma_start(p4[:], v4[:, s, :])
                p1 = T([P, CHUNK, D1], "p1")
                nc.sync.dma_start(p1[:], v1[:, s, :])
                p2 = T([P, CHUNK, D2], "p2")
                nc.sync.dma_start(p2[:], v2[:, s, :])

                feat = T([P, CHUNK, NFG, D_FEAT], "feat")

                # ---------- softmax scores + threshold mask -> d16 (DVE+ACT) ----------
                lt = p1[:, :, 0:4]
                sb = T([P, CHUNK, C], "sb")
                nc.scalar.activation(sb[:], lt, EXP)
                sm = T([P, CHUNK], "sm")
                nc.vector.tensor_reduce(sm[:], sb[:], AX, OP.add)
                nc.vector.reciprocal(sm[:], sm[:])
                sc = T([P, CHUNK, NFG], "sc")
                nc.vector.tensor_tensor(
                    sc[:],
                    sb[:, :, 1:C],
                    sm[:, :, None].to_broadcast([P, CHUNK, NFG]),
                    OP.mult,
                )
                nc.vector.scalar_tensor_tensor(
                    feat[:, :, :, 16], sc[:], SCORE_THRESH, sc[:], OP.is_gt, OP.mult
                )

                # ---------- dims: exp(hwl) * mean  (all on ACT) ----------
                exh = T([P, CHUNK, C, 3], "exh")
                nc.scalar.activation(
                    exh[:], p4[:, :, 48:60].rearrange("p f (c k) -> p f c k", c=C), EXP
                )
                for d in range(3):
                    nc.scalar.activation(
                        feat[:, :, :, 12 + d], exh[:, :, 1:C, d], CPY,
                        scale=MEAN_DIMS[d],
                    )

                # ---------- boxes (DVE+ACT) + centers (GPSIMD), both sides batched ----------
                # pack4 layout: bbox_l 0:16 | bbox_r 16:32 | ctr_l 32:40 | ctr_r 40:48
                code = p4[:, :, 0:32].rearrange("p f (s c k) -> p f s c k", s=2, c=C)
                ctr = p4[:, :, 32:48].rearrange("p f (s c k) -> p f s c k", s=2, c=C)
                SH3 = [P, CHUNK, 2, NFG]
                featb = feat[:, :, :, 0:8].rearrange("p f c (s k) -> p f s c k", s=2)
                featc = feat[:, :, :, 8:12].rearrange("p f c (s k) -> p f s c k", s=2)
                w01 = wh01[:, s, :, 0][:, :, :, None].to_broadcast(SH3)
                h01 = wh01[:, s, :, 1][:, :, :, None].to_broadcast(SH3)
                whf = whh[:, s, :, 0][:, :, :, None].to_broadcast(SH3)
                hhf = whh[:, s, :, 1][:, :, :, None].to_broadcast(SH3)
                cxb = cxy[:, s, :, 0][:, :, :, None].to_broadcast(SH3)
                cyb = cxy[:, s, :, 1][:, :, :, None].to_broadcast(SH3)

                # pcx = code0 * (0.1*w) + cx ; pcy analogous  (both sides at once)
                pcx = T(SH3, "pcx")
                nc.vector.tensor_tensor(pcx[:], code[:, :, :, 1:C, 0], w01, OP.mult)
                nc.vector.tensor_tensor(pcx[:], pcx[:], cxb, OP.add)
                pcy = T(SH3, "pcy")
                nc.vector.tensor_tensor(pcy[:], code[:, :, :, 1:C, 1], h01, OP.mult)
                nc.vector.tensor_tensor(pcy[:], pcy[:], cyb, OP.add)

                # hpw = min(exp(code2*0.2), CLAMP) * (0.5*w)
                hpw = T(SH3, "hpw")
                nc.scalar.activation(hpw[:], code[:, :, :, 1:C, 2], EXP, scale=0.2)
                nc.vector.tensor_scalar_min(hpw[:], hpw[:], EXP_CLAMP)
                nc.vector.tensor_tensor(hpw[:], hpw[:], whf, OP.mult)
                hph = T(SH3, "hph")
                nc.scalar.activation(hph[:], code[:, :, :, 1:C, 3], EXP, scale=0.2)
                nc.vector.tensor_scalar_min(hph[:], hph[:], EXP_CLAMP)
                nc.vector.tensor_tensor(hph[:], hph[:], hhf, OP.mult)

                x1t = T(SH3, "x1t")
                nc.vector.tensor_tensor(x1t[:], pcx[:], hpw[:], OP.subtract)
                nc.vector.tensor_scalar(
                    featb[:, :, :, :, 0], x1t[:], 0.0, IMG_W - 1, OP.max, OP.min
                )
                y1t = T(SH3, "y1t")
                nc.vector.tensor_tensor(y1t[:], pcy[:], hph[:], OP.subtract)
                nc.vector.tensor_scalar(
                    featb[:, :, :, :, 1], y1t[:], 0.0, IMG_H - 1, OP.max, OP.min
                )
                x2t = T(SH3, "x2t")
                nc.vector.tensor_tensor(x2t[:], pcx[:], hpw[:], OP.add)
                nc.vector.tensor_scalar(x2t[:], x2t[:], 1.0, 0.0, OP.subtract, OP.max)
                nc.vector.tensor_scalar_min(featb[:, :, :, :, 2], x2t[:], IMG_W - 1)
                y2t = T(SH3, "y2t")
                nc.vector.tensor_tensor(y2t[:], pcy[:], hph[:], OP.add)
                nc.vector.tensor_scalar(y2t[:], y2t[:], 1.0, 0.0, OP.subtract, OP.max)
                nc.vector.tensor_scalar_min(featb[:, :, :, :, 3], y2t[:], IMG_H - 1)

                # centers -> feat d8..11 (GPSIMD)
                cdx = T(SH3, "cdx")
                nc.vector.tensor_tensor(cdx[:], ctr[:, :, :, 1:C, 0], w01, OP.mult)
                nc.vector.tensor_tensor(featc[:, :, :, :, 0], cdx[:], cxb, OP.add)
                cdy = T(SH3, "cdy")
                nc.vector.tensor_tensor(cdy[:], ctr[:, :, :, 1:C, 1], h01, OP.mult)
                nc.vector.tensor_tensor(featc[:, :, :, :, 1], cdy[:], cyb, OP.add)

                # ---------- rotation (one eq*reg pass; mult on GPSIMD) ----------
                alt = p1[:, :, 4:14]
                mxa = T([P, CHUNK], "mxa")
                nc.vector.tensor_reduce(mxa[:], alt, AX, OP.max)
                eq = T([P, CHUNK, B], "eq")
                nc.vector.tensor_tensor(
                    eq[:], alt, mxa[:, :, None].to_broadcast([P, CHUNK, B]), OP.is_equal
                )
                rrt = T([P, CHUNK, C, B], "rrt")
                nc.vector.tensor_tensor(
                    rrt[:],
                    eq[:, :, None, :].to_broadcast([P, CHUNK, C, B]),
                    p2[:, :, :].rearrange("p f (c b) -> p f c b", c=C),
                    OP.mult,
                )
                rr4 = T([P, CHUNK, C], "rr4")
                nc.vector.tensor_reduce(rr4[:], rrt[:], AX, OP.add)
                rsum = T([P, CHUNK, NFG], "rsum")
                nc.vector.tensor_tensor(
                    rsum[:],
                    rr4[:, :, 0][:, :, None].to_broadcast([P, CHUNK, NFG]),
                    rr4[:, :, 1:C],
                    OP.add,
                )
                nc.vector.tensor_scalar(
                    feat[:, :, :, 15], rsum[:], BIN_SIZE, -PI_F32, OP.mult, OP.add
                )

                nc.sync.dma_start(vout[:, s, :, :], feat[:])

    return nc
_NC_CACHE = None


def _get_nc():
    global _NC_CACHE
    if _NC_CACHE is None:
        nc = _build_nc()
        nc.compile()
        _NC_CACHE = nc
    return _NC_CACHE


def _iou_row(b, boxes, areas):
    """reference's iou(): one box b vs array of boxes [K,4] (float32)."""
    ix1 = np.maximum(boxes[:, 0], b[0])
    iy1 = np.maximum(boxes[:, 1], b[1])
    ix2 = np.minimum(boxes[:, 2], b[2])
    iy2 = np.minimum(boxes[:, 3], b[3])
    f32 = np.float32
    iw = np.maximum((ix2 - ix1) + f32(1.0), f32(0.0))
    ih = np.maximum((iy2 - iy1) + f32(1.0), f32(0.0))
    inter = iw * ih
    barea = ((b[2] - b[0]) + f32(1.0)) * ((b[3] - b[1]) + f32(1.0))
    return inter / ((areas + barea) - inter)


def _host_finish(feats):
    """feats: [N, NFG, 17] float32 device output -> [100, 17] final result."""
    f32 = np.float32
    flat_scores = np.full(NFG * MAX_PER_CLASS, NEG, dtype=f32)
    flat_feats = np.zeros((NFG * MAX_PER_CLASS, 16), dtype=f32)

    for ci in range(NFG):
        s = feats[:, ci, 16]
        cand = np.flatnonzero(s > SCORE_THRESH)
        if cand.size:
            # score desc, index asc (argmax-tie semantics)
            order = cand[np.lexsort((cand, -s[cand].astype(np.float64)))]
        else:
            order = cand
        bl = feats[:, ci, 0:4]
        br = feats[:, ci, 4:8]
        kept = []
        kept_bl = np.empty((MAX_PER_CLASS, 4), dtype=f32)
        kept_br = np.empty((MAX_PER_CLASS, 4), dtype=f32)
        kept_al = np.empty(MAX_PER_CLASS, dtype=f32)
        kept_ar = np.empty(MAX_PER_CLASS, dtype=f32)
        for i in order:
            if len(kept) >= MAX_PER_CLASS:
                break
            nk = len(kept)
            if nk:
                iou_l = _iou_row(bl[i], kept_bl[:nk], kept_al[:nk])
                iou_r = _iou_row(br[i], kept_br[:nk], kept_ar[:nk])
                if np.maximum(iou_l, iou_r).max() > NMS_THR:
                    continue
            kept_bl[nk] = bl[i]
            kept_br[nk] = br[i]
            kept_al[nk] = ((bl[i, 2] - bl[i, 0]) + f32(1.0)) * (
                (bl[i, 3] - bl[i, 1]) + f32(1.0)
            )
            kept_ar[nk] = ((br[i, 2] - br[i, 0]) + f32(1.0)) * (
                (br[i, 3] - br[i, 1]) + f32(1.0)
            )
            kept.append(i)

        base = ci * MAX_PER_CLASS
        nk = len(kept)
        if nk:
            ki = np.asarray(kept)
            flat_scores[base : base + nk] = s[ki]
            flat_feats[base : base + nk] = feats[ki, ci, 0:16]
        # keep == -1 slots: score NEG, features of proposal 0 (safe index 0)
        if nk < MAX_PER_CLASS:
            flat_feats[base + nk : base + MAX_PER_CLASS] = feats[0, ci, 0:16]

    # global top-100: score desc, flat index asc
    top = np.lexsort(
        (np.arange(flat_scores.size), -flat_scores.astype(np.float64))
    )[:DETS_PER_IMG]
    top_s = flat_scores[top]
    valid = top_s > f32(NEG * 0.5)
    mask = valid.astype(f32)
    out = np.empty((DETS_PER_IMG, D_FEAT), dtype=f32)
    out[:, 0:16] = flat_feats[top] * mask[:, None]
    out[:, 16] = np.where(valid, top_s, f32(0.0))
    return out


def _pack_inputs(inputs):
    pack1 = np.empty((N, D1), dtype=np.float32)
    pack1[:, 0:4] = inputs["class_logits"]
    pack1[:, 4:14] = inputs["alpha_logit"]
    pack2 = np.array(inputs["alpha_reg"], dtype=np.float32, copy=True)
    pack2[:, 0:10] = np.arange(B, dtype=np.float32)
    pack3 = np.empty((N, D3), dtype=np.float32)
    pack3[:, 0:4] = inputs["proposals_left"]
    pack3[:, 4:8] = inputs["proposals_right"]
    pack4 = np.empty((N, D4), dtype=np.float32)
    pack4[:, 0:16] = inputs["bbox_reg_left"]
    pack4[:, 16:32] = inputs["bbox_reg_right"]
    pack4[:, 32:40] = inputs["center_reg_left"]
    pack4[:, 40:48] = inputs["center_reg_right"]
    pack4[:, 48:60] = inputs["hwl_reg"]
    return pack1, pack2, pack3, pack4


def _run_device(inputs, **spmd_kwargs):
    nc = _get_nc()
    packs = _pack_inputs(inputs)
    in_maps = []
    for c in range(NCORES):
        sl = slice(c * NS, (c + 1) * NS)
        in_maps.append(
            {f"pack{i + 1}": p[sl] for i, p in enumerate(packs)}
        )
    res = run_bass_kernel_spmd(nc, in_maps, list(range(NCORES)), **spmd_kwargs)
    feats = np.concatenate(
        [np.asarray(res.results[c]["feat"]) for c in range(NCORES)], axis=0
    )
    return feats, res


def kernel(**inputs):
    try:
        feats, _ = _run_device(inputs)
    except Exception:
        # transient NRT execution failures have been observed to succeed on
        # retry (device recovers between runs)
        import time as _time

        _time.sleep(5.0)
        feats, _ = _run_device(inputs)
    return _host_finish(feats)

